# revision 24
# baseline (speedup 1.0000x reference)
"""Multi-head attention (B=2, S=4096, D=512, H=8) on 8 TRN2 NeuronCores.

Sharding: (batch, head-pair) tensor parallel. Core i handles batch i//4
and heads 2*(i%4), 2*(i%4)+1: Q/K/V projections for its two heads, full
S x S attention, and a partial output projection. The host sums the 4
partials per batch (f32) and adds bo once -- no device collectives.

The baseline bottleneck was softmax exp: 256 ACT instructions x ~1.09us
= 278us busy on one engine. This version splits exp across TWO engines
and runs attn@V in fp8 DoubleRow (contraction 256 = two 128-k-chunks
per matmul), roughly halving its PE time:

  1. x^T is pre-transposed on the host; plain segmented DMA loads.
  2. Q^T/K^T = W x^T bf16, both heads row-packed [128, 4096].
     V [t, dv] in fp8e4 V_aug [128t, (pair,j,head), 65] with a ones
     column per head (softmax denominator rides the attn@V matmul).
  3. Per (q-tile 512, k-chunk 128): 2 row-tiled score matmuls (c=64,
     heads at PE rows 0-63/64-127 run concurrently) -> psum [128,2,512].
     exp: EVEN chunks on ACT (exact exp, fp8e4-convert out); ODD chunks
     on DVE as a single tensor_scalar (score*log2e + 56.05 -> uint8)
     that constructs the e4m3 BIT PATTERN directly (Schraudolph in fp8
     space; the constant is tuned for zero multiplicative bias vs exact
     exp, rel-err contribution ~3% rms on attention weights, ~1e-3 on
     the output). One psum->fp8 crossing per element at 1 elem/lane/cyc
     is the hard floor on both engines; splitting is the only win.
  4. attn@V: per (chunk-PAIR, head) one DoubleRow fp8 matmul
     lhsT=[128,2,65] V_aug pair, rhs=[128,2,512] p pair, accumulating
     po [65, 1024] psum (both heads + denominators in one 2-bank tile).
  5. Normalize per q-tile: po row 64 -> [1,1024] reciprocal_approx_fast
     -> fp16 -> two rank-1 broadcast matmuls -> one scalar_tensor_tensor
     -> aot bf16; output projection per 256-t chunk, f32 DMA out.

Scheduling: one shared 3x2-bank psum rotation for scores + proj/fin/pb2
accumulators (a 2-buffer scores pool serializes on the exp->scores WAR
chain), po 1x2 banks. Projection/fin work drains into the attention
loop's PE slack as weighted pending units with verified deadlines
(k_unit(tt) before scores(4tt) at iter 2tt-1, v chunks before their
attn@V pair). The qtile boundary frees po with four parallel two-engine
copies; reciprocal/normalize/fin are deferred off the critical path.

Steady state: all three engines ~22us busy per 26.8us q-tile (ACT 17
exps + copies, DVE 15 exps + recip/stt/v-bias, PE scores+attnV+drained
projections). HW ~268us vs 325us baseline; rel err 1.2e-2 (gate 2e-2).
"""

import numpy as np
import ml_dtypes

import concourse.bass as bass
import concourse.tile as tile
from concourse import bacc, mybir
from concourse.bass_utils import run_bass_kernel_spmd

F32 = mybir.dt.float32
FP16 = mybir.dt.float16
F32R = mybir.dt.float32r
BF16 = mybir.dt.bfloat16
FP8 = mybir.dt.float8e4
U8 = mybir.dt.uint8
MUL = mybir.AluOpType.mult
ADD = mybir.AluOpType.add
DR = mybir.MatmulPerfMode.DoubleRow

B, S, D, H = 2, 4096, 512, 8
HD = D // H  # 64
NCORES = 8
PAIRS = 4  # head-pairs; one per core (per batch)
IC = D // 128  # 4 contraction chunks over d_model
QT = 512  # q tile
NQT = S // QT  # 8
KCH = S // 128  # 32 k chunks
NCP = KCH // 2  # 16 chunk pairs (DoubleRow contracts 2 chunks)
SEG = 1024  # t-columns per transposed DMA segment
NSEG = S // SEG  # 4

LOG2E = 1.4426950408889634
# e4m3 Schraudolph bias: 56 (exponent bias*8) + sawtooth centering +0.5
# for the truncating f32->uint8 convert. Tuned numerically for zero
# multiplicative bias vs the exact-exp path (see session notes).
SCHRAU_C = 56.05


def _build_program():
    nc = bacc.Bacc(
        "TRN2",
        target_bir_lowering=False,
        debug=False,
        enable_asserts=False,
        num_devices=NCORES,
    )
    xt = nc.dram_tensor("xt", [D, S], BF16, kind="ExternalInput").ap()
    wqt = nc.dram_tensor("wqt", [D, 128], BF16, kind="ExternalInput").ap()
    wkt = nc.dram_tensor("wkt", [D, 128], BF16, kind="ExternalInput").ap()
    wvt = nc.dram_tensor("wvt", [D, 128], BF16, kind="ExternalInput").ap()
    wos = nc.dram_tensor("wos", [128, D], BF16, kind="ExternalInput").ap()
    bqs = nc.dram_tensor("bqs", [128, 1], F32, kind="ExternalInput").ap()
    bks = nc.dram_tensor("bks", [128, 1], F32, kind="ExternalInput").ap()
    bvb = nc.dram_tensor("bvb", [128, 128], F32, kind="ExternalInput").ap()
    out = nc.dram_tensor("out", [S, D], F32, kind="ExternalOutput").ap()

    with tile.TileContext(nc) as tc:
        with (
            tc.tile_pool(name="consts", bufs=1) as consts,
            tc.tile_pool(name="persist", bufs=1) as persist,
            tc.tile_pool(name="pt", bufs=3) as pt_pool,
            tc.tile_pool(name="aot", bufs=2) as aot_pool,
            tc.tile_pool(name="osb", bufs=4) as osb_pool,
            tc.tile_pool(name="posb", bufs=2) as posb_pool,
            tc.tile_pool(name="small", bufs=4) as small_pool,
            # PSUM (8 banks): one shared 3x2-bank rotation for scores +
            # proj/fin/pb2 accumulators (breaks the exp->scores WAR chain
            # that a 2-buffer scores pool serializes on), po 1x2 banks.
            tc.tile_pool(name="ps", bufs=3, space="PSUM") as ps_pool,
            tc.tile_pool(name="ps_po", bufs=1, space="PSUM") as po_pool,
        ):
            # ---- constants ----
            ones64f = consts.tile([1, HD], F32)
            nc.vector.memset(ones64f, 1.0)
            ones64 = consts.tile([1, HD], FP16)
            nc.vector.tensor_copy(ones64, ones64f)

            # ---- persistent activations ----
            xtks = [
                persist.tile([128, IC, SEG], BF16, name=f"xtk{s}")
                for s in range(NSEG)
            ]
            kt = persist.tile([128, S], BF16)  # K^T pair [dv, t]
            qt = persist.tile([128, S], BF16)  # Q^T pair
            # V_aug fp8: flat dim = (cp, j, h); 80-padded rows, col 64 = ones
            v2 = persist.tile([128, KCH * 2, 80], FP8)
            nc.vector.memset(v2[:, :, 64:65], 1.0)
            v2r = v2.rearrange("p (cp j h) m -> p cp j h m", cp=NCP, j=2, h=2)

            # ---- DMAs (x^T pre-transposed on host; plain loads) ----
            xtd = xt.rearrange("(c p) t -> p c t", p=128)
            wq_sb = consts.tile([128, IC, 128], BF16)
            nc.sync.dma_start(wq_sb, wqt.rearrange("(c p) o -> p c o", p=128))
            bq_sb = consts.tile([128, 1], F32)
            nc.sync.dma_start(bq_sb, bqs)
            wk_sb = consts.tile([128, IC, 128], BF16)
            nc.sync.dma_start(wk_sb, wkt.rearrange("(c p) o -> p c o", p=128))
            bk_sb = consts.tile([128, 1], F32)
            nc.sync.dma_start(bk_sb, bks)
            nc.sync.dma_start(xtks[0][:, :, 0:512], xtd[:, :, 0:512])
            wv_sb = consts.tile([128, IC, 128], BF16)
            nc.sync.dma_start(wv_sb, wvt.rearrange("(c p) o -> p c o", p=128))
            bvb_sb = consts.tile([128, 128], F32)
            nc.sync.dma_start(bvb_sb, bvb)
            nc.sync.dma_start(xtks[0][:, :, 512:SEG], xtd[:, :, 512:SEG])
            wo_sb = consts.tile([128, D], BF16)
            nc.sync.dma_start(wo_sb, wos)
            for s in range(1, NSEG):
                nc.sync.dma_start(xtks[s], xtd[:, :, s * SEG : (s + 1) * SEG])

            # ---- projection units ----
            def q_unit(tt):
                ps = ps_pool.tile([128, QT], F32, tag="sc", name=f"q{tt}")
                s, ss = divmod(tt, 2)
                for i in range(IC):
                    nc.tensor.matmul(
                        ps,
                        wq_sb[:, i, :],
                        xtks[s][:, i, ss * QT : (ss + 1) * QT],
                        start=(i == 0),
                        stop=(i == IC - 1),
                    )
                nc.scalar.add(qt[:, tt * QT : (tt + 1) * QT], ps, bq_sb[:, 0:1])

            def k_unit(tt, lo=0, hi=QT):
                ps = ps_pool.tile(
                    [128, hi - lo], F32, tag="sc", name=f"k{tt}_{lo}"
                )
                s, ss = divmod(tt, 2)
                for i in range(IC):
                    nc.tensor.matmul(
                        ps,
                        wk_sb[:, i, :],
                        xtks[s][:, i, ss * QT + lo : ss * QT + hi],
                        start=(i == 0),
                        stop=(i == IC - 1),
                    )
                nc.vector.tensor_scalar_add(
                    kt[:, tt * QT + lo : tt * QT + hi], ps, bk_sb[:, 0:1]
                )

            def v_unit(j):
                # V rows for t-chunk j, both heads: [128 t, 128 dv] + bias
                ps = ps_pool.tile([128, 128], F32, tag="sc", name=f"v{j}")
                s, jj = divmod(j, 8)
                for i in range(IC):
                    nc.tensor.matmul(
                        ps,
                        xtks[s][:, i, jj * 128 : (jj + 1) * 128],
                        wv_sb[:, i, :],
                        start=(i == 0),
                        stop=(i == IC - 1),
                    )
                cp, pj = divmod(j, 2)
                nc.vector.tensor_add(
                    v2r[:, cp, pj, :, 0:64],
                    ps.rearrange("p (h d) -> p h d", h=2),
                    bvb_sb.rearrange("p (h d) -> p h d", h=2),
                )

            # upfront: bare minimum for scores(0)/attnV(0) -- q-tile 0,
            # the first 128 kt columns, V chunks 0-1. Everything else
            # drains into the loop's slack as (weight, fn) units.
            q_unit(0)
            k_unit(0, 0, 128)
            v_unit(0)
            v_unit(1)
            pending = [(1, lambda: k_unit(1))]
            for g in range(1, NCP):
                if g % 2 == 0:  # k2..k7 at g2,g4,..,g12
                    tt = g // 2 + 1
                    if tt <= 7:
                        pending.append((1, lambda tt=tt: k_unit(tt)))
                c0 = 2 * g
                pending.append((1, lambda j=c0: v_unit(j)))
                pending.append((1, lambda j=c0 + 1: v_unit(j)))
            for tt in range(1, 8):
                pending.append((1, lambda tt=tt: q_unit(tt)))

            pending_slow = []

            aots = {}

            def rec_unit(qi, db, recf, recs, last=False):
                nc.vector.reciprocal_approx_fast(recf, db)
                if last:
                    nc.scalar.copy(recs, recf)
                else:
                    nc.vector.tensor_copy(recs, recf)

            def norm_unit(qi, recs):
                pb2 = ps_pool.tile([128, QT], F32, tag="sc", name=f"pb{qi}")
                nc.tensor.matmul(
                    pb2[0:HD, :], ones64, recs[:, 0:QT], start=True, stop=True
                )
                nc.tensor.matmul(
                    pb2[HD:128, :], ones64, recs[:, QT : 2 * QT],
                    start=True, stop=True,
                )
                nc.vector.scalar_tensor_tensor(
                    aots[qi], pb2, 1.0, posbs[qi], op0=MUL, op1=MUL
                )

            outr = out.rearrange("(c p) d -> p c d", p=128)

            def fin_unit(qi, t2):
                # two 128-t output chunks per psum tile (halves the number
                # of 4KB insertions into the shared psum rotation)
                ps = ps_pool.tile(
                    [128, 2, D], F32, tag="sc", name=f"f{qi}_{t2}"
                )
                for u in range(2):
                    nc.tensor.matmul(
                        ps[:, u, :],
                        aots[qi][:, (2 * t2 + u) * 128 : (2 * t2 + u + 1) * 128],
                        wo_sb,
                        start=True,
                        stop=True,
                    )
                osb = osb_pool.tile([128, 2, D], F32, tag="osb")
                nc.scalar.copy(osb, ps)
                c0 = qi * 4 + 2 * t2
                nc.sync.dma_start(outr[:, c0 : c0 + 2, :], osb)

            posbs = {}
            po_drain = [None]

            # ---- attention ----
            for qi in range(NQT):
                qs = qi * QT
                aots[qi] = aot_pool.tile(
                    [128, QT], BF16, tag="aot", name=f"aot{qi}"
                )
                # po: one single-bank tile per head so each head's bank
                # frees independently at the qtile boundary (next qtile's
                # attn_v head h waits only on head h's two drain readers)
                po_a = po_pool.tile([65, QT], F32, tag="poa", name=f"poa{qi}")
                po_b = po_pool.tile([65, QT], F32, tag="pob", name=f"pob{qi}")
                pos = (po_a, po_b)

                def scores(k, qs=qs):
                    pss = ps_pool.tile([128, 2, QT], F32, tag="sc")
                    for hh in range(2):
                        off = hh * HD
                        nc.tensor.matmul(
                            pss[:, hh, :],
                            kt[off : off + HD, k * 128 : (k + 1) * 128],
                            qt[off : off + HD, qs : qs + QT],
                            start=True,
                            stop=True,
                        )
                    return pss

                ptiles = {}

                def exp(k, pss):
                    cp, j = divmod(k, 2)
                    if j == 0:
                        ptiles[cp] = pt_pool.tile(
                            [128, 2, 2, QT], FP8, tag="pt", name=f"pt{cp}"
                        )
                    pt = ptiles[cp]
                    if k % 2 == 0 or k == 19:
                        # ACT: exact exp, fp8e4 convert
                        nc.scalar.activation(
                            pt[:, :, j, :], pss,
                            mybir.ActivationFunctionType.Exp, scale=0.125,
                        )
                    else:
                        # DVE: e4m3 bit-pattern exp (Schraudolph)
                        nc.vector.tensor_scalar(
                            pt[:, :, j, :].bitcast(U8), pss,
                            LOG2E, SCHRAU_C, op0=MUL, op1=ADD,
                        )

                def attn_v(cp, pos=pos):
                    pt = ptiles.pop(cp)
                    for hh in range(2):
                        nc.tensor.matmul(
                            pos[hh],
                            v2r[:, cp, :, hh, 0:65],
                            pt[:, hh, :, :],
                            start=(cp == 0),
                            stop=(cp == NCP - 1),
                            perf_mode=DR,
                        )

                # software pipeline: scores/exp run ahead of attn@V
                pss = scores(0)
                exp(0, pss)
                if qi == 0:
                    k_unit(0, 128, QT)  # kt chunks 1-3, before scores(1)
                pss = scores(1)
                exp(1, pss)
                # drain the PREVIOUS qtile's po only now -- after this
                # qtile's first two exps are queued on ACT/DVE -- so
                # attn_v(0) (blocked on the po banks) unblocks while the
                # exps are already done rather than queued behind these.
                if po_drain[0] is not None:
                    po_drain[0]()
                    po_drain[0] = None
                for cp in range(NCP):
                    it = qi * NCP + cp
                    # drain deferred projection / norm / fin work FIRST so
                    # their engine-queue entries precede the scores/attn@V
                    # that consume them (k_unit(tt) must precede the
                    # lookahead scores(4tt) below; v_unit(j) must precede
                    # attn_v(j//2)).
                    heavy = total = 0
                    while pending and heavy < 3 and total < 6:
                        w, fn = pending.pop(0)
                        fn()
                        heavy += w
                        total += 1
                    if not pending and pending_slow and it % 2 == 1:
                        pending_slow.pop(0)()
                    for k in (2 * cp + 2, 2 * cp + 3):
                        if k < KCH:
                            pss = scores(k)
                            exp(k, pss)
                    attn_v(cp)

                # free po fast: the four po-readers run two-per-engine in
                # parallel; the reciprocal is deferred off the boundary.
                db = small_pool.tile([1, 2 * QT], F32, tag="db")
                posbN = posb_pool.tile(
                    [128, QT], F32, tag="posb", name=f"posb{qi}"
                )

                def drain_po(qi=qi, pos=pos, db=db, posbN=posbN):
                    # each engine reads head0's bank FIRST so po_a frees in
                    # one op-time on both engines
                    nc.scalar.copy(db[:, 0:QT], pos[0][64:65, :])
                    nc.vector.tensor_copy(posbN[0:HD, :], pos[0][0:HD, :])
                    nc.vector.tensor_copy(db[:, QT : 2 * QT], pos[1][64:65, :])
                    nc.scalar.copy(posbN[HD:128, :], pos[1][0:HD, :])

                posbs[qi] = posbN
                recf = small_pool.tile([1, 2 * QT], F32, tag="recf")
                recs = small_pool.tile([1, 2 * QT], FP16, tag="rec")
                last = qi == NQT - 1
                if last:
                    drain_po()
                    rec_unit(qi, db, recf, recs, last=True)
                else:
                    po_drain[0] = drain_po
                    pending_slow.append(
                        lambda qi=qi, db=db, recf=recf, recs=recs: rec_unit(
                            qi, db, recf, recs
                        )
                    )
                pending_slow.append(lambda qi=qi, recs=recs: norm_unit(qi, recs))
                pending_slow.extend(
                    lambda qi=qi, t2=t2: fin_unit(qi, t2) for t2 in range(2)
                )

            for u in pending + pending_slow:
                u()

    nc.compile()
    return nc


_NC_CACHE = None


def _get_program():
    global _NC_CACHE
    if _NC_CACHE is None:
        _NC_CACHE = _build_program()
    return _NC_CACHE


def prepare_in_maps(x, Wq, bq, Wk, bk, Wv, bv, Wo, bo):
    bf = ml_dtypes.bfloat16
    x = np.ascontiguousarray(np.asarray(x, dtype=np.float32)).astype(bf)
    wqT = np.asarray(Wq, np.float32).T  # [D in, D out-rows]
    wkT = np.asarray(Wk, np.float32).T
    wvT = np.asarray(Wv, np.float32).T
    woT = np.asarray(Wo, np.float32).T  # [D dv, D out]
    bq = np.asarray(bq, np.float32)
    bk = np.asarray(bk, np.float32)
    bv = np.asarray(bv, np.float32)
    in_maps = []
    for core in range(NCORES):
        b = core // PAIRS
        hp = core % PAIRS
        pr = slice(hp * 128, (hp + 1) * 128)
        m = {
            "xt": np.ascontiguousarray(x[b].T),
            "wqt": np.ascontiguousarray(wqT[:, pr]).astype(bf),
            "wkt": np.ascontiguousarray(wkT[:, pr]).astype(bf),
            "wvt": np.ascontiguousarray(wvT[:, pr]).astype(bf),
            "wos": np.ascontiguousarray(woT[pr, :]).astype(bf),
            "bqs": np.ascontiguousarray(bq[pr].reshape(128, 1)),
            "bks": np.ascontiguousarray(bk[pr].reshape(128, 1)),
            "bvb": np.ascontiguousarray(
                np.broadcast_to(bv[pr][None, :], (128, 128))
            ),
        }
        in_maps.append(m)
    return in_maps


def assemble(results, bo):
    out = np.empty((B, S, D), dtype=np.float32)
    bo = np.asarray(bo, np.float32)
    for b in range(B):
        acc = results[b * PAIRS]["out"].astype(np.float32, copy=True)
        for hp in range(1, PAIRS):
            acc += results[b * PAIRS + hp]["out"]
        out[b] = acc + bo[None, :]
    return out


def kernel(x, Wq, bq, Wk, bk, Wv, bv, Wo, bo):
    in_maps = prepare_in_maps(x, Wq, bq, Wk, bk, Wv, bv, Wo, bo)
    nc = _get_program()
    res = run_bass_kernel_spmd(nc, in_maps, core_ids=list(range(NCORES)))
    return assemble(res.results, bo)
